# revision 1
# baseline (speedup 1.0000x reference)
"""MultiHeadAttention Trainium2 kernel (8-core SPMD, head/batch sharded).

Reference semantics (E=1024, H=16, D=64, B=2, S=2048):
    qp = (q @ wq.T + bq).reshape(B, H, S, D)   # RAW view, not transpose!
    scores = qp @ kp^T * 1/sqrt(E); attn = softmax(scores)
    out = (attn @ vp).reshape(B, S, E) @ wo.T + bo

Because the reshape is a raw view, head h of batch b corresponds to the
contiguous 128-row block rows[128h:128h+128] of the projected [S, E]
matrix, viewed as [2048, 64].  Each core therefore only needs 512 rows of
q/k/v (4 heads) plus the full weight matrices.

Inside each head we use the permuted sequence order i' = 128r + a
(original in-head index i = 16a + r, a=row-in-block 0..127, r=col-block
0..15).  This is a symmetric permutation of Q/K/V rows, so softmax+AV
commute with it; it makes every layout matmul-native:
  * qT/kT tiles: [64(d), 16(r), 128(a)] per head — produced directly by
    feature-block projection matmuls (features n = 128v + p*64 + d).
  * V stays in natural row layout; AV uses V as the stationary operand
    with an appended ones column so softmax denominators ride along as
    output row 64.
All matmuls run in bf16 with fp32 PSUM accumulation (bf16 weight loads use
FWL + the background weight buffer, so LDWEIGHTS hides behind the streams;
f32r fuses the load inline and measured ~1.5x slower per matmul).
"""

import numpy as np

import concourse.bass as bass
import concourse.mybir as mybir
import concourse.tile as tile
from concourse import bacc
from concourse.bass_utils import run_bass_kernel_spmd

B, S, E = 2, 2048, 1024
H, D = 16, 64
HEADS_PER_CORE = 4
ROWS = 512  # rows of the [S,E] projected matrix handled per core
N_CORES = 8
SCALE = 1.0 / float(np.sqrt(np.float32(E)))

F32 = mybir.dt.float32
F32R = mybir.dt.float32r  # (f32r matmuls fuse LDWEIGHTS inline -> slow; bf16 everywhere)
BF16 = mybir.dt.bfloat16
I16 = mybir.dt.int16
AF = mybir.ActivationFunctionType

# IEEE bit-trick exp2 in bf16: exp(SCALE*x) ~= bits_as_bf16(AEXP*x + BEXP).
# (bf16 has a 7-bit mantissa; the int16 pattern fits comfortably.)
LOG2E = 1.4426950408889634
AEXP = float(2**7 * LOG2E) * SCALE
BEXP = float(2**7 * (127 - 0.0434609) + 0.5)

# Fraction (out of 16) of exp chunks offloaded from ScalarE to VectorE bit-exp.
DVE_EXP_NUM = 5


def use_dve_exp(c, half):
    idx = 2 * c + half
    return (idx * DVE_EXP_NUM) % 16 < DVE_EXP_NUM


def build_nc():
    nc = bacc.Bacc(
        "TRN2",
        target_bir_lowering=False,
        debug=False,
        num_devices=N_CORES,
    )

    # DRAM parameters (per-core shapes; host passes per-core slices).
    # x* are transposed+augmented on host: [1025, 512], row 1024 = ones.
    # w* are w.T augmented with the bias as row 1024: [1025, 1024].
    xq = nc.dram_tensor("xq", [E + 1, ROWS], BF16, kind="ExternalInput").ap()
    xk = nc.dram_tensor("xk", [E + 1, ROWS], BF16, kind="ExternalInput").ap()
    xv = nc.dram_tensor("xv", [E + 1, ROWS], BF16, kind="ExternalInput").ap()
    wq = nc.dram_tensor("wq", [E + 1, E], BF16, kind="ExternalInput").ap()
    wk = nc.dram_tensor("wk", [E + 1, E], BF16, kind="ExternalInput").ap()
    wv = nc.dram_tensor("wv", [E + 1, E], BF16, kind="ExternalInput").ap()
    wo = nc.dram_tensor("wo", [E + 1, E], BF16, kind="ExternalInput").ap()
    y = nc.dram_tensor("y", [ROWS, E], F32, kind="ExternalOutput").ap()

    with tile.TileContext(nc) as tc:
        build_tile_kernel(tc, xq, xk, xv, wq, wk, wv, wo, y)

    nc.compile()
    return nc


def load_w(pool, nc, wdram, name):
    """DMA a [1024, 1024] weight into two [128, 4, 1024] chunked half-tiles.

    Two half-tiles on a bufs=3 tag let the next phase's first half prefetch
    while the current phase still holds its second half.
    """
    halves = []
    for i in range(2):
        w_sb = pool.tile([128, 4, E], BF16, tag="w", name=f"{name}{i}")
        nc.sync.dma_start(
            out=w_sb,
            in_=wdram[512 * i : 512 * i + 512, :].rearrange("(c p) e -> p c e", p=128),
        )
        halves.append(w_sb)
    return halves


def wslice(w_halves, k, cols):
    return w_halves[k // 4][:, k % 4, cols]


def load_x(pool, nc, xdram, name):
    x_sb = pool.tile([128, 8, ROWS], BF16, tag="x", name=name)
    # gpsimd (SWDGE) queue so x loads run parallel to the sync-queue weights
    nc.gpsimd.dma_start(
        out=x_sb, in_=xdram[0:E, :].rearrange("(c p) m -> p c m", p=128)
    )
    return x_sb


def build_tile_kernel(tc, xq, xk, xv, wq, wk, wv, wo, y):
    nc = tc.nc

    with (
        tc.tile_pool(name="persist", bufs=1) as persist,
        tc.tile_pool(name="wpool", bufs=4) as wpool,
        tc.tile_pool(name="xpool", bufs=2) as xpool,
        tc.tile_pool(name="small", bufs=1) as small,
        tc.tile_pool(name="expp", bufs=4) as expp,
        tc.tile_pool(name="tailp", bufs=2) as tailp,
        tc.tile_pool(name="outp", bufs=2) as outp,
        tc.tile_pool(name="dramp", bufs=2, space="DRAM") as dramp,
        # one 2-bank tag serves both the projection accumulators ([128,512])
        # and the attention AV accumulators ([65,1024]); sc gets the other 4.
        tc.tile_pool(name="pacc", bufs=2, space="PSUM") as pacc,
        tc.tile_pool(name="psc", bufs=2, space="PSUM") as psc,
    ):
        pav = pacc
        # ---------------- persistent SBUF tensors ----------------
        # qT/kT: [128, pair, r, a]; head h lives at partitions 64*(h%2)..+64,
        # pair index h//2.  Value at [64*(h%2)+d, h//2, r, a] = proj[128h+a, 64r+d].
        qT = persist.tile([128, 2, 16, 128], BF16)
        kT = persist.tile([128, 2, 16, 128], BF16)
        # vones per head: [128(a), 16(r), 65]; [...,:64] = vp rows, [...,64] = 1.0
        # bf16: the AV matmul runs in bf16 (same PE rate, no f32r rounding rule)
        vones = [
            persist.tile([128, 16, D + 1], BF16, tag=f"vones{h}", name=f"vones{h}")
            for h in range(4)
        ]
        # oT: attention output, transposed for the out-projection:
        # [128(e%128), 8(e//128), 512(m)]  where e = 64r+d, m = 128h+a.
        oT = persist.tile([128, 8, ROWS], BF16)
        ones_col = small.tile([1, 128], BF16)  # lhsT ones row for out-proj bias
        nc.vector.memset(ones_col, 1.0)
        for h in range(4):
            nc.vector.memset(vones[h][:, :, D : D + 1], 1.0)

        # big first-phase loads go first on the sync (HWDGE) queue
        xq_sb = load_x(xpool, nc, xq, "xq_sb")
        wq_sb = load_w(wpool, nc, wq, "wq_sb")

        # bias rows (row 1024 of the augmented weights) — tiny DMAs on the
        # otherwise-idle gpsimd (SWDGE) queue so they don't delay the weights
        bq = small.tile([1, E], BF16, tag="bq")
        bk = small.tile([1, E], BF16, tag="bk")
        bv = small.tile([1, E], BF16, tag="bv")
        bo = small.tile([1, E], BF16, tag="bo")
        nc.gpsimd.dma_start(out=bq, in_=wq[E : E + 1, :])
        nc.gpsimd.dma_start(out=bk, in_=wk[E : E + 1, :])
        nc.gpsimd.dma_start(out=bv, in_=wv[E : E + 1, :])
        nc.gpsimd.dma_start(out=bo, in_=wo[E : E + 1, :])
        xq_ones = small.tile([1, ROWS], BF16, tag="xqo")
        xk_ones = small.tile([1, ROWS], BF16, tag="xko")
        xv_ones = small.tile([1, ROWS], BF16, tag="xvo")
        nc.gpsimd.dma_start(out=xq_ones, in_=xq[E : E + 1, :])
        nc.gpsimd.dma_start(out=xk_ones, in_=xk[E : E + 1, :])
        nc.gpsimd.dma_start(out=xv_ones, in_=xv[E : E + 1, :])

        # ---------------- Q / K projections (transposed layout) ----------
        proj_transposed(tc, pacc, small, wq_sb, xq_sb, bq, xq_ones, qT, "q")
        xk_sb = load_x(xpool, nc, xk, "xk_sb")
        wk_sb = load_w(wpool, nc, wk, "wk_sb")
        proj_transposed(tc, pacc, small, wk_sb, xk_sb, bk, xk_ones, kT, "k")

        # ---------------- V projection (natural layout into vones) -------
        xv_sb = load_x(xpool, nc, xv, "xv_sb")
        wv_sb = load_w(wpool, nc, wv, "wv_sb")
        for h in range(4):
            # g inner so each stationary xv block serves both 512-wide groups
            accs = [
                pacc.tile([128, 512], F32, tag="accum", name=f"accv{h}{g}")
                for g in range(2)
            ]
            for k in range(9):
                for g in range(2):
                    if k < 8:
                        lhsT = xv_sb[:, k, 128 * h : 128 * h + 128]
                        rhs = wslice(wv_sb, k, slice(512 * g, 512 * g + 512))
                    else:
                        lhsT = xv_ones[:, 128 * h : 128 * h + 128]
                        rhs = bv[:, 512 * g : 512 * g + 512]
                    nc.tensor.matmul(accs[g], lhsT, rhs, start=(k == 0), stop=(k == 8))
            for g in range(2):
                nc.vector.tensor_copy(
                    vones[h][:, 8 * g : 8 * g + 8, 0:D],
                    accs[g].rearrange("p (rr d) -> p rr d", d=D),
                )

        # ---------------- attention, head pairs ----------------
        wo_sb = load_w(wpool, nc, wo, "wo_sb")  # prefetch during attention
        for pr in range(2):
            attention_pair(tc, psc, pav, expp, tailp, dramp, qT, kT, vones, oT, pr)

        # ---------------- output projection ----------------
        for mb in range(4):
            y_sb = outp.tile([128, E], F32, tag="ysb", name=f"ysb{mb}")
            accs = [
                pacc.tile([128, 512], F32, tag="accum", name=f"accy{mb}{g}")
                for g in range(2)
            ]
            for v in range(9):
                for g in range(2):
                    if v < 8:
                        lhsT = oT[:, v, 128 * mb : 128 * mb + 128]
                        rhs = wslice(wo_sb, v, slice(512 * g, 512 * g + 512))
                    else:
                        lhsT = ones_col
                        rhs = bo[:, 512 * g : 512 * g + 512]
                    nc.tensor.matmul(accs[g], lhsT, rhs, start=(v == 0), stop=(v == 8))
            for g in range(2):
                nc.vector.tensor_copy(y_sb[:, 512 * g : 512 * g + 512], accs[g])
            nc.sync.dma_start(out=y[128 * mb : 128 * mb + 128, :], in_=y_sb)


def proj_transposed(tc, pacc, small, w_sb, x_sb, bias, xones, dst, nm):
    """Project x @ w.T into the per-head transposed layout `dst`.

    Feature-block v of the PSUM output holds features n = 128v + 64p + d at
    partition 64p + d (p = upper/lower half), i.e. r = 2v + p.  Head h wants
    its data at partition half h%2, so blocks with p == h%2 copy straight
    through (VectorE) and the other half bounce via a staging tile and two
    partition-shifting SBUF->SBUF DMAs.
    """
    nc = tc.nc
    stg = small.tile([128, 8, 2, 128], BF16, tag="stg", name=f"stg_{nm}")
    for v in range(8):
        acc = pacc.tile([128, 512], F32, tag="accum", name=f"acc{nm}{v}")
        for k in range(9):
            if k < 8:
                lhsT = wslice(w_sb, k, slice(128 * v, 128 * v + 128))
                rhs = x_sb[:, k, :]
            else:
                lhsT = bias[:, 128 * v : 128 * v + 128]
                rhs = xones
            nc.tensor.matmul(acc, lhsT, rhs, start=(k == 0), stop=(k == 8))
        src = acc.rearrange("p (h a) -> p h a", a=128)
        for p in range(2):
            # heads with h%2 == p whose data sits in psum half q:
            #   q == p   -> direct copy to dst[64p:64p+64, :, 2v+p, :]
            #   q == 1-p -> staging (partition-shift later via DMA)
            direct = src[64 * p : 64 * p + 64, p::2, :]
            nc.vector.tensor_copy(dst[64 * p : 64 * p + 64, :, 2 * v + p, :], direct)
            q = 1 - p
            mismatched = src[64 * q : 64 * q + 64, p::2, :]
            nc.vector.tensor_copy(stg[64 * q : 64 * q + 64, v, :, :], mismatched)
    for pr in range(2):
        # staged upper half (q=1): r = 2v+1 data for even-parity heads -> lower dst half
        nc.sync.dma_start(
            out=dst[0:64, pr, 1::2, :], in_=stg[64:128, :, pr, :]
        )
        # staged lower half (q=0): r = 2v data for odd-parity heads -> upper dst half
        nc.sync.dma_start(
            out=dst[64:128, pr, 0::2, :], in_=stg[0:64, :, pr, :]
        )


def attention_pair(tc, psc, pav, expp, tailp, dramp, qT, kT, vones, oT, pr):
    """Process heads (2*pr, 2*pr+1) together.

    The two heads live at partition halves 0/1 of the qT/kT tiles, so their
    QK matmuls land on disjoint PE row-strips ((0,0) vs (64,0)) and run
    concurrently.  Per chunk one head's exp runs on ScalarE (true exp) and
    the other's on VectorE (bit-trick exp2, ~2% elementwise, washes out in
    softmax averaging); alternation by chunk keeps both heads 50/50.
    """
    nc = tc.nc
    for ih in range(2):  # i' half: columns 1024*ih .. 1024*ih+1024
        # rows 0..63: attention output (transposed); row 64: softmax denom
        av = [
            pav.tile([D + 1, 1024], F32, tag="accum", name=f"av{pr}{ih}{half}")
            for half in range(2)
        ]
        for c in range(16):
            sc = [
                psc.tile([128, 1024], F32, tag="sc", name=f"sc{pr}{ih}{c}{half}")
                for half in range(2)
            ]
            for gg in range(2):
                for half in range(2):  # interleave so row-strips pair up
                    base = 64 * half
                    nc.tensor.matmul(
                        sc[half][:, 512 * gg : 512 * gg + 512],
                        kT[base : base + 64, pr, c, :],
                        qT[base : base + 64, pr, 8 * ih + 4 * gg : 8 * ih + 4 * gg + 4, :],
                        start=True,
                        stop=True,
                        tile_position=(base, 0),
                    )
            ex = []
            for half in range(2):
                if not use_dve_exp(c, half):
                    e = expp.tile(
                        [128, 1024], BF16, tag="ex", name=f"ex{pr}{ih}{c}{half}"
                    )
                    nc.scalar.activation(e, sc[half], AF.Exp, scale=SCALE)
                else:
                    ei = expp.tile(
                        [128, 1024], I16, tag="ex", name=f"exi{pr}{ih}{c}{half}"
                    )
                    nc.vector.tensor_scalar(
                        out=ei,
                        in0=sc[half],
                        scalar1=AEXP,
                        scalar2=BEXP,
                        op0=mybir.AluOpType.mult,
                        op1=mybir.AluOpType.add,
                    )
                    e = ei.bitcast(BF16)
                ex.append(e)
            for half in range(2):
                h = 2 * pr + half
                for gg in range(2):
                    nc.tensor.matmul(
                        av[half][:, 512 * gg : 512 * gg + 512],
                        vones[h][:, c, :],
                        ex[half][:, 512 * gg : 512 * gg + 512],
                        start=(c == 0),
                        stop=(c == 15),
                    )

        # Release both PSUM accumulators FIRST (copies are the only readers of
        # `av`), then run the reciprocal/normalize chains off the critical
        # path on GPSIMD + DMA so neither the PE nor the DVE exp stream
        # stalls behind them.
        rel = [tail_release(tc, tailp, av[half], 2 * pr + half, ih) for half in range(2)]
        for half in range(2):
            tail_finish(tc, tailp, dramp, *rel[half], oT, 2 * pr + half, ih)


def tail_release(tc, tailp, av, h, ih):
    """Drain the PSUM accumulator immediately so its slot frees for the next
    head pair: ScalarE takes the denominator row, VectorE the numerators."""
    nc = tc.nc
    den_sb = tailp.tile([D + 1, 1024], F32, tag="rec", name=f"den{h}{ih}")
    nc.scalar.copy(den_sb[D : D + 1, :], av[D : D + 1, :])
    # ScalarE (not DVE): the Tile scheduler reorders the DVE FIFO by priority
    # and was parking this release copy behind the previous tail's reciprocal,
    # holding the PSUM slot ~8us and stalling the next pair's AV matmuls.
    avb = tailp.tile([64, 1024], F32, tag="avb", name=f"avb{h}{ih}")
    nc.scalar.copy(avb, av[0:64])
    return den_sb, avb


def tail_finish(tc, tailp, dramp, den_sb, avb, oT, h, ih):
    """Reciprocal + normalize + scatter into oT, entirely on GPSIMD/DMA/DVE.

    DVE reciprocal cost scales with FREE size (~6.4ns/elem), so reshape the
    1024 denominators to [8, 128] via a DRAM bounce (coarse 512B runs),
    reciprocal there, and broadcast back.  The multiplies run on GpSimd
    (SBUF-only inputs) to keep the DVE queue clear for the exp stream.
    """
    nc = tc.nc
    den_d = dramp.tile([1, 1024], F32, tag="dend", name=f"dend{h}{ih}")
    nc.gpsimd.dma_start(out=den_d, in_=den_sb[D : D + 1, :])
    den_t = tailp.tile([8, 128], F32, tag="dent", name=f"dent{h}{ih}")
    nc.gpsimd.dma_start(
        out=den_t, in_=den_d.rearrange("o (t a) -> (o t) a", t=8)
    )
    nc.vector.reciprocal(den_t, den_t)
    rec_d = dramp.tile([1, 1024], F32, tag="recd", name=f"recd{h}{ih}")
    nc.gpsimd.dma_start(out=rec_d.rearrange("o (t a) -> (o t) a", t=8), in_=den_t)
    rec_bc = tailp.tile([64, 1024], F32, tag="recbc", name=f"recbc{h}{ih}")
    nc.gpsimd.dma_start(out=rec_bc, in_=rec_d.partition_broadcast(64))

    av_r = avb.rearrange("d (rh two a) -> d rh two a", two=2, a=128)
    bc_r = rec_bc.rearrange("d (rh two a) -> d rh two a", two=2, a=128)
    # even r (= 8*ih + 2*rh): partitions already correct (e%128 = d)
    nc.gpsimd.tensor_tensor(
        out=oT[0:64, 4 * ih : 4 * ih + 4, 128 * h : 128 * h + 128],
        in0=av_r[:, :, 0, :],
        in1=bc_r[:, :, 0, :],
        op=mybir.AluOpType.mult,
    )
    # odd r: normalize into staging, then partition-shift DMA into oT[64:128]
    stg_o = tailp.tile([64, 4, 128], BF16, tag="stgo", name=f"stgo{h}{ih}")
    nc.gpsimd.tensor_tensor(
        out=stg_o,
        in0=av_r[:, :, 1, :],
        in1=bc_r[:, :, 1, :],
        op=mybir.AluOpType.mult,
    )
    nc.gpsimd.dma_start(
        out=oT[64:128, 4 * ih : 4 * ih + 4, 128 * h : 128 * h + 128], in_=stg_o
    )


_NC_CACHE = {}


def get_nc():
    if "nc" not in _NC_CACHE:
        _NC_CACHE["nc"] = build_nc()
    return _NC_CACHE["nc"]


def shard_inputs(q, k, v, wq, bq, wk, bk, wv, bv, wo, bo):
    """Build the 8 per-core input maps (host-side transposes/augments)."""

    import ml_dtypes

    bf16 = ml_dtypes.bfloat16

    def aug_w(w, b):
        return np.concatenate(
            [np.ascontiguousarray(np.asarray(w, np.float32).T),
             np.asarray(b, np.float32)[None, :]],
            axis=0,
        ).astype(bf16)

    wq_a, wk_a = aug_w(wq, bq), aug_w(wk, bk)
    wv_a, wo_a = aug_w(wv, bv), aug_w(wo, bo)
    ones = np.ones((1, ROWS), np.float32)

    in_maps = []
    for c in range(N_CORES):
        b = c // 4
        r0 = 512 * (c % 4)
        sl = slice(r0, r0 + ROWS)

        def aug_x(x):
            xt = np.ascontiguousarray(np.asarray(x[b, sl, :], np.float32).T)
            return np.concatenate([xt, ones], axis=0).astype(bf16)

        in_maps.append(
            {
                "xq": aug_x(q),
                "xk": aug_x(k),
                "xv": aug_x(v),
                "wq": wq_a,
                "wk": wk_a,
                "wv": wv_a,
                "wo": wo_a,
            }
        )
    return in_maps


def assemble_output(results):
    out = np.empty((B, S, E), np.float32)
    for c in range(N_CORES):
        b = c // 4
        r0 = 512 * (c % 4)
        out[b, r0 : r0 + ROWS, :] = results[c]["y"]
    return out


def kernel(q, k, v, wq, bq, wk, bk, wv, bv, wo, bo, **run_kwargs):
    nc = get_nc()
    in_maps = shard_inputs(q, k, v, wq, bq, wk, bk, wv, bv, wo, bo)
    res = run_bass_kernel_spmd(nc, in_maps, list(range(N_CORES)), **run_kwargs)
    out = assemble_output(res.results)
    if run_kwargs:
        return out, res
    return out

